# revision 2
# baseline (speedup 1.0000x reference)
"""Trainium2 Bass kernel for nn_Blur: depthwise 4x4 FIR conv, pad=2. v2.

out[b,c,h',w'] = sum_{i,j} wf[i,j] * xpad[b,c,h'+i,w'+j],  wf = flip(kernel)
x: [8,256,256,256] f32, kernel: [4,4] f32 -> out: [8,256,257,257] f32

Pure data parallel over batch (8 cores). The host pre-builds, per core,
a padded h-major tensor xp[H, C, 262] (2 zero | 256 data | 4 zero per
channel row) matching the SBUF tile layout exactly, so every load
partition line is ONE contiguous 16.8KB descriptor (CB=16 channels).
Output rows 0..249 store h-major [250, C, 257] (contiguous 16.4KB per
partition line); the 7-row tail is computed block-diagonally packed (16
channels in one matmul: K=16x8=128, M=16x7=112) and stored packed to a
side tensor that the host unscrambles. 2112 matmuls/core total.
"""

import numpy as np

_C, _H, _W = 256, 256, 256
_HO, _WO = 257, 257
_NCORES = 8
# v0/v1 per-channel tiles: (hp0, Mv, hlo, Kv)
_TILES = [(0, 125, 0, 126), (125, 125, 123, 128)]
# v2 packed tail: out rows 250..256 (7), x rows 248..255 (8), 16 ch/matmul
_V2_HP0, _V2_M, _V2_HLO, _V2_K = 250, 7, 248, 8
_NW = 262  # padded width: 2 zero | 256 data | 4 zero
_NMM = 258  # matmul free dim (257 outputs + 1 garbage col), must be even
_CB = 16  # channels per DMA batch
_NG = _C // _CB


def _build_bands(kern):
    wf = np.ascontiguousarray(np.asarray(kern, np.float32)[::-1, ::-1])
    bands = np.zeros((128, 2, 4, 125), np.float32)
    for v, (hp0, Mv, hlo, Kv) in enumerate(_TILES):
        for j in range(4):
            for hr in range(Kv):
                h = hlo + hr
                for mr in range(Mv):
                    i = h - (hp0 + mr) + 2
                    if 0 <= i < 4:
                        bands[hr, v, j, mr] = wf[i, j]
    # block-diagonal tail bands: partition p = hr*16 + c, out m = c*7 + r
    bands2 = np.zeros((128, 4, 112), np.float32)
    for j in range(4):
        for hr in range(_V2_K):
            for c in range(_CB):
                for r in range(_V2_M):
                    i = hr - r  # (248+hr) - (250+r) + 2
                    if 0 <= i < 4:
                        bands2[hr * _CB + c, j, c * _V2_M + r] = wf[i, j]
    return bands, bands2


_NC_CACHE = {}


def _build_nc():
    if "nc" in _NC_CACHE:
        return _NC_CACHE["nc"]
    import concourse.bacc as bacc
    import concourse.mybir as mybir
    import concourse.tile as tile

    nc = bacc.Bacc()
    x_d = nc.declare_dram_parameter("x", [_H, _C, _NW], mybir.dt.float32r, isOutput=False)
    b_d = nc.declare_dram_parameter(
        "bands", [128, 2, 4, 125], mybir.dt.float32r, isOutput=False
    )
    b2_d = nc.declare_dram_parameter(
        "bands2", [128, 4, 112], mybir.dt.float32r, isOutput=False
    )
    o_d = nc.declare_dram_parameter("out", [250, _C, _WO], mybir.dt.float32, isOutput=True)
    o2_d = nc.declare_dram_parameter(
        "out2", [_NG, _CB * _V2_M, _WO], mybir.dt.float32, isOutput=True
    )
    xv2_d = nc.declare_dram_parameter(
        "xv2", [_NG, 128, _NW], mybir.dt.float32r, isOutput=False
    )

    NBX = 2  # ring depth per v-tile stream
    NBO = 2
    with tile.TileContext(nc) as tc:
        with (
            tc.tile_pool(name="sb", bufs=1) as pool,
            tc.tile_pool(name="ps", bufs=1, space="PSUM") as pp,
        ):
            band_sb = pool.tile([128, 2, 4, 125], mybir.dt.float32r, tag="bands")
            nc.sync.dma_start(out=band_sb[:], in_=b_d[:])
            band2_sb = pool.tile([128, 4, 112], mybir.dt.float32r, tag="bands2")
            nc.sync.dma_start(out=band2_sb[:], in_=b2_d[:])

            xts = {
                (v, i): pool.tile(
                    [128, _CB, _NW], mybir.dt.float32r,
                    tag=f"xt{v}_{i}", name=f"xt{v}_{i}",
                )
                for v in range(2)
                for i in range(NBX)
            }
            xv2s = [
                pool.tile([128, _NW], mybir.dt.float32r, tag=f"xv2_{i}", name=f"xv2_{i}")
                for i in range(NBX)
            ]
            oss = {
                (v, i): pool.tile(
                    [128, _CB, _WO], mybir.dt.float32,
                    tag=f"os{v}_{i}", name=f"os{v}_{i}",
                )
                for v in range(2)
                for i in range(NBO)
            }
            os2s = [
                pool.tile([128, _WO], mybir.dt.float32, tag=f"os2_{i}", name=f"os2_{i}")
                for i in range(NBO)
            ]
            pss = [
                pp.tile([128, _NMM], mybir.dt.float32, tag=f"ps{i}", name=f"ps{i}")
                for i in range(8)
            ]
            ps2 = pp.tile([128, _NMM], mybir.dt.float32, tag="ps2", name="ps2")

            unit = 0  # global (ch, v) unit counter for psum/copy-engine rotation
            for gi in range(_NG):
                c0 = gi * _CB
                ri = gi % NBX
                for v, (hp0, Mv, hlo, Kv) in enumerate(_TILES):
                    nc.sync.dma_start(
                        out=xts[(v, ri)][0:Kv, :, :],
                        in_=x_d[hlo : hlo + Kv, c0 : c0 + _CB, :],
                    )
                nc.sync.dma_start(out=xv2s[ri][0:128, :], in_=xv2_d[gi, :, :])
                for v, (hp0, Mv, hlo, Kv) in enumerate(_TILES):
                    xt = xts[(v, ri)]
                    osb = oss[(v, gi % NBO)]
                    for ch in range(_CB):
                        ps = pss[unit % 8]
                        for j in range(4):
                            nc.tensor.matmul(
                                ps[0:Mv, 0:_NMM],
                                band_sb[0:Kv, v, j, 0:Mv],
                                xt[0:Kv, ch, j : j + _NMM],
                                start=(j == 0),
                                stop=(j == 3),
                            )
                        if unit % 2 == 0:
                            nc.vector.tensor_copy(osb[0:Mv, ch, 0:_WO], ps[0:Mv, 0:_WO])
                        else:
                            nc.scalar.copy(osb[0:Mv, ch, 0:_WO], ps[0:Mv, 0:_WO])
                        unit += 1
                    nc.gpsimd.dma_start(
                        out=o_d[hp0 : hp0 + Mv, c0 : c0 + _CB, :],
                        in_=osb[0:Mv, :, :],
                    )
                # packed tail tile: psum rows (c, r); stored packed, host unscrambles
                osb2 = os2s[gi % NBO]
                for j in range(4):
                    nc.tensor.matmul(
                        ps2[0 : _CB * _V2_M, 0:_NMM],
                        band2_sb[0:128, j, 0 : _CB * _V2_M],
                        xv2s[ri][0:128, j : j + _NMM],
                        start=(j == 0),
                        stop=(j == 3),
                    )
                if gi % 2 == 0:
                    nc.vector.tensor_copy(
                        osb2[0 : _CB * _V2_M, 0:_WO], ps2[0 : _CB * _V2_M, 0:_WO]
                    )
                else:
                    nc.scalar.copy(
                        osb2[0 : _CB * _V2_M, 0:_WO], ps2[0 : _CB * _V2_M, 0:_WO]
                    )
                nc.gpsimd.dma_start(
                    out=o2_d[gi, :, :], in_=osb2[0 : _CB * _V2_M, 0:_WO]
                )
    nc.finalize()
    _NC_CACHE["nc"] = nc
    return nc


def _run(x, kern, trace=False):
    from concourse.bass_utils import run_bass_kernel_spmd

    x = np.asarray(x, dtype=np.float32)
    bands, bands2 = _build_bands(kern)
    nc = _build_nc()
    in_maps = []
    for b in range(_NCORES):
        xp = np.zeros((_H, _C, _NW), np.float32)
        xp[:, :, 2:258] = x[b].transpose(1, 0, 2)
        # packed tail input: partition p = h*16 + c_local per 16-channel group
        xv2 = np.ascontiguousarray(
            xp[_V2_HLO : _V2_HLO + _V2_K]
            .reshape(_V2_K, _NG, _CB, _NW)
            .transpose(1, 0, 2, 3)
            .reshape(_NG, 128, _NW)
        )
        in_maps.append({"x": xp, "bands": bands, "bands2": bands2, "xv2": xv2})
    res = run_bass_kernel_spmd(nc, in_maps, list(range(_NCORES)), trace=trace)
    out = np.empty((_NCORES, _C, _HO, _WO), np.float32)
    for b in range(_NCORES):
        o = np.asarray(res.results[b]["out"])  # [250, C, 257]
        out[b, :, 0:250, :] = o.transpose(1, 0, 2)
        o2 = np.asarray(res.results[b]["out2"]).reshape(_NG, _CB, _V2_M, _WO)
        out[b].reshape(_NG, _CB, _HO, _WO)[:, :, 250:257, :] = o2
    return out, res


def kernel(x, kernel):
    out, _ = _run(x, kernel, trace=False)
    return out


# revision 3
# speedup vs baseline: 1.0215x; 1.0215x over previous
"""Trainium2 Bass kernel for nn_Blur: depthwise 4x4 FIR conv, pad=2. v2.

out[b,c,h',w'] = sum_{i,j} wf[i,j] * xpad[b,c,h'+i,w'+j],  wf = flip(kernel)
x: [8,256,256,256] f32, kernel: [4,4] f32 -> out: [8,256,257,257] f32

Pure data parallel over batch (8 cores). The host pre-builds, per core,
a padded h-major tensor xp[H, C, 262] (2 zero | 256 data | 4 zero per
channel row) matching the SBUF tile layout exactly, so every load
partition line is ONE contiguous 16.8KB descriptor (CB=16 channels).
Output rows 0..249 store h-major [250, C, 257] (contiguous 16.4KB per
partition line); the 7-row tail is computed block-diagonally packed (16
channels in one matmul: K=16x8=128, M=16x7=112) and stored packed to a
side tensor that the host unscrambles. 2112 matmuls/core total.
"""

import ml_dtypes
import numpy as np

_BF16 = ml_dtypes.bfloat16

_C, _H, _W = 256, 256, 256
_HO, _WO = 257, 257
_NCORES = 8
# v0/v1 per-channel tiles: (hp0, Mv, hlo, Kv)
_TILES = [(0, 125, 0, 126), (125, 125, 123, 128)]
# v2 packed tail: out rows 250..256 (7), x rows 248..255 (8), 16 ch/matmul
_V2_HP0, _V2_M, _V2_HLO, _V2_K = 250, 7, 248, 8
_NW = 262  # padded width: 2 zero | 256 data | 4 zero
_NMM = 258  # matmul free dim (257 outputs + 1 garbage col), must be even
_CB = 16  # channels per DMA batch
_NG = _C // _CB


def _build_bands(kern):
    wf = np.ascontiguousarray(np.asarray(kern, np.float32)[::-1, ::-1])
    bands = np.zeros((128, 2, 4, 125), np.float32)
    for v, (hp0, Mv, hlo, Kv) in enumerate(_TILES):
        for j in range(4):
            for hr in range(Kv):
                h = hlo + hr
                for mr in range(Mv):
                    i = h - (hp0 + mr) + 2
                    if 0 <= i < 4:
                        bands[hr, v, j, mr] = wf[i, j]
    # block-diagonal tail bands: partition p = hr*16 + c, out m = c*7 + r
    bands2 = np.zeros((128, 4, 112), np.float32)
    for j in range(4):
        for hr in range(_V2_K):
            for c in range(_CB):
                for r in range(_V2_M):
                    i = hr - r  # (248+hr) - (250+r) + 2
                    if 0 <= i < 4:
                        bands2[hr * _CB + c, j, c * _V2_M + r] = wf[i, j]
    return bands, bands2


_NC_CACHE = {}


def _build_nc():
    if "nc" in _NC_CACHE:
        return _NC_CACHE["nc"]
    import concourse.bacc as bacc
    import concourse.mybir as mybir
    import concourse.tile as tile

    nc = bacc.Bacc()
    x_d = nc.declare_dram_parameter("x", [_H, _C, _NW], mybir.dt.bfloat16, isOutput=False)
    b_d = nc.declare_dram_parameter(
        "bands", [128, 2, 4, 125], mybir.dt.bfloat16, isOutput=False
    )
    b2_d = nc.declare_dram_parameter(
        "bands2", [128, 4, 112], mybir.dt.bfloat16, isOutput=False
    )
    o_d = nc.declare_dram_parameter("out", [250, _C, _WO], mybir.dt.bfloat16, isOutput=True)
    o2_d = nc.declare_dram_parameter(
        "out2", [_NG, _CB * _V2_M, _WO], mybir.dt.bfloat16, isOutput=True
    )
    xv2_d = nc.declare_dram_parameter(
        "xv2", [_NG, 128, _NW], mybir.dt.bfloat16, isOutput=False
    )

    NBX = 3  # ring depth per v-tile stream
    NBO = 3
    with tile.TileContext(nc) as tc:
        with (
            tc.tile_pool(name="sb", bufs=1) as pool,
            tc.tile_pool(name="ps", bufs=1, space="PSUM") as pp,
        ):
            band_sb = pool.tile([128, 2, 4, 125], mybir.dt.bfloat16, tag="bands")
            nc.sync.dma_start(out=band_sb[:], in_=b_d[:])
            band2_sb = pool.tile([128, 4, 112], mybir.dt.bfloat16, tag="bands2")
            nc.sync.dma_start(out=band2_sb[:], in_=b2_d[:])

            xts = {
                (v, i): pool.tile(
                    [128, _CB, _NW], mybir.dt.bfloat16,
                    tag=f"xt{v}_{i}", name=f"xt{v}_{i}",
                )
                for v in range(2)
                for i in range(NBX)
            }
            xv2s = [
                pool.tile([128, _NW], mybir.dt.bfloat16, tag=f"xv2_{i}", name=f"xv2_{i}")
                for i in range(NBX)
            ]
            oss = {
                (v, i): pool.tile(
                    [128, _CB, _WO], mybir.dt.bfloat16,
                    tag=f"os{v}_{i}", name=f"os{v}_{i}",
                )
                for v in range(2)
                for i in range(NBO)
            }
            os2s = [
                pool.tile([128, _WO], mybir.dt.bfloat16, tag=f"os2_{i}", name=f"os2_{i}")
                for i in range(NBO)
            ]
            pss = [
                pp.tile([128, _NMM], mybir.dt.float32, tag=f"ps{i}", name=f"ps{i}")
                for i in range(8)
            ]
            ps2 = pp.tile([128, _NMM], mybir.dt.float32, tag="ps2", name="ps2")

            unit = 0  # global (ch, v) unit counter for psum/copy-engine rotation
            for gi in range(_NG):
                c0 = gi * _CB
                ri = gi % NBX
                for v, (hp0, Mv, hlo, Kv) in enumerate(_TILES):
                    nc.sync.dma_start(
                        out=xts[(v, ri)][0:Kv, :, :],
                        in_=x_d[hlo : hlo + Kv, c0 : c0 + _CB, :],
                    )
                nc.sync.dma_start(out=xv2s[ri][0:128, :], in_=xv2_d[gi, :, :])
                for v, (hp0, Mv, hlo, Kv) in enumerate(_TILES):
                    xt = xts[(v, ri)]
                    osb = oss[(v, gi % NBO)]
                    for ch in range(_CB):
                        ps = pss[unit % 8]
                        for j in range(4):
                            nc.tensor.matmul(
                                ps[0:Mv, 0:_NMM],
                                band_sb[0:Kv, v, j, 0:Mv],
                                xt[0:Kv, ch, j : j + _NMM],
                                start=(j == 0),
                                stop=(j == 3),
                            )
                        if unit % 2 == 0:
                            nc.vector.tensor_copy(osb[0:Mv, ch, 0:_WO], ps[0:Mv, 0:_WO])
                        else:
                            nc.scalar.copy(osb[0:Mv, ch, 0:_WO], ps[0:Mv, 0:_WO])
                        unit += 1
                    st = nc.gpsimd if (gi * 2 + v) % 2 == 0 else nc.scalar
                    st.dma_start(
                        out=o_d[hp0 : hp0 + Mv, c0 : c0 + _CB, :],
                        in_=osb[0:Mv, :, :],
                    )
                # packed tail tile: psum rows (c, r); stored packed, host unscrambles
                osb2 = os2s[gi % NBO]
                for j in range(4):
                    nc.tensor.matmul(
                        ps2[0 : _CB * _V2_M, 0:_NMM],
                        band2_sb[0:128, j, 0 : _CB * _V2_M],
                        xv2s[ri][0:128, j : j + _NMM],
                        start=(j == 0),
                        stop=(j == 3),
                    )
                if gi % 2 == 0:
                    nc.vector.tensor_copy(
                        osb2[0 : _CB * _V2_M, 0:_WO], ps2[0 : _CB * _V2_M, 0:_WO]
                    )
                else:
                    nc.scalar.copy(
                        osb2[0 : _CB * _V2_M, 0:_WO], ps2[0 : _CB * _V2_M, 0:_WO]
                    )
                st2 = nc.gpsimd if gi % 2 == 0 else nc.scalar
                st2.dma_start(
                    out=o2_d[gi, :, :], in_=osb2[0 : _CB * _V2_M, 0:_WO]
                )
    nc.finalize()
    _NC_CACHE["nc"] = nc
    return nc


def _run(x, kern, trace=False):
    from concourse.bass_utils import run_bass_kernel_spmd

    x = np.asarray(x, dtype=np.float32)
    bands, bands2 = _build_bands(kern)
    bands = bands.astype(_BF16)
    bands2 = bands2.astype(_BF16)
    nc = _build_nc()
    in_maps = []
    for b in range(_NCORES):
        xp = np.zeros((_H, _C, _NW), _BF16)
        xp[:, :, 2:258] = x[b].transpose(1, 0, 2).astype(_BF16)
        # packed tail input: partition p = h*16 + c_local per 16-channel group
        xv2 = np.ascontiguousarray(
            xp[_V2_HLO : _V2_HLO + _V2_K]
            .reshape(_V2_K, _NG, _CB, _NW)
            .transpose(1, 0, 2, 3)
            .reshape(_NG, 128, _NW)
        )
        in_maps.append({"x": xp, "bands": bands, "bands2": bands2, "xv2": xv2})
    res = run_bass_kernel_spmd(nc, in_maps, list(range(_NCORES)), trace=trace)
    out = np.empty((_NCORES, _C, _HO, _WO), np.float32)
    for b in range(_NCORES):
        o = np.asarray(res.results[b]["out"]).astype(np.float32)  # [250, C, 257]
        out[b, :, 0:250, :] = o.transpose(1, 0, 2)
        o2 = np.asarray(res.results[b]["out2"]).astype(np.float32).reshape(_NG, _CB, _V2_M, _WO)
        out[b].reshape(_NG, _CB, _HO, _WO)[:, :, 250:257, :] = o2
    return out, res


def kernel(x, kernel):
    out, _ = _run(x, kernel, trace=False)
    return out
